# revision 23
# baseline (speedup 1.0000x reference)
"""Trainium2 Bass kernel for nn_AttentionLayer (segment softmax attention pooling).

Computation (reference):
    h = tanh(x @ W1 + b1)            # [N, A]
    s = h @ W2 + b2                  # [N, 1]
    per-segment softmax over s, out[b] = sum_i softmax_w_i * x_i   # [B, D]

Strategy (v2, bf16 data path + grouped exp):
  - Shard the N=500k instances across 8 NeuronCores (data parallel), weights
    replicated. Host pre-converts x to bf16 and pre-transposes so each core
    streams xT [D=128, rows] tiles (bf16 halves HBM traffic and doubles PE
    throughput vs the f32r baseline).
  - Per core, one pass over x, 2048-row chunks, software-pipelined:
      PE : hT = W1^T @ xT                     (4x N=512 matmuls, bf16)
      ACT: th = tanh(hT + b1) -> bf16
      PE : 4x col-tiled score matmuls -> sg [128, 512] grouped scores
           (col group g holds scores for rows 512g..512g+511 on partitions
           32g..32g+31; ACT cost is free-dim-driven, so exp on the grouped
           layout is 4x cheaper than on the broadcast layout)
      ACT: eg = exp(sg + b2) -> bf16 [128, 512]
      PE : 4x K=1 ones-matmuls broadcast eg group rows -> ebc [128, 2048]
           (PSUM) = e_i on every partition
      DVE: affine_mul_reduce(xT * ebc) summed per 1024-row window -> wacc
  - Device outputs: per-window weighted sums wacc [D, NWIN] and the e row
    (bf16, via the grouped eg tile).  Segment logic on the host: pure windows
    used directly; windows containing a segment boundary recomputed from x
    and the exported e; denominators via bincount over e.  exp without max
    subtraction is safe (scores O(+-5)); numerator/denominator share e.
"""

import numpy as np

# Problem constants (hardcoded per contract; kernel.py must be self-contained).
N = 500_000
D = 128
A = 128
B = 256
NCORES = 8
RPC = N // NCORES            # rows per core = 62500
CHUNK = 2048                 # rows per streamed tile
WIN = 1024                   # rows per reduction window
G = CHUNK // 4               # grouped score tile free dim = 512
NCHUNK = -(-RPC // CHUNK)    # 31
RPAD = NCHUNK * CHUNK        # 63488
NWIN = RPAD // WIN           # 62
MM_N = 512                   # PE moving-operand max free dim

_prog_cache = {}


def _build_program():
    import concourse.bacc as bacc
    from concourse import mybir
    from concourse.tile import TileContext

    f32 = mybir.dt.float32
    bf16 = mybir.dt.bfloat16
    nc = bacc.Bacc("TRN2", target_bir_lowering=False, debug=False,
                   num_devices=NCORES)

    xt = nc.dram_tensor("xt", [D, RPAD], bf16, kind="ExternalInput")
    w1 = nc.dram_tensor("w1", [D, A], bf16, kind="ExternalInput")
    w2r = nc.dram_tensor("w2r", [A, 128], bf16, kind="ExternalInput")
    ones1 = nc.dram_tensor("ones1", [128, 128], bf16, kind="ExternalInput")
    b1 = nc.dram_tensor("b1", [A, 1], f32, kind="ExternalInput")
    b2 = nc.dram_tensor("b2", [128, 1], f32, kind="ExternalInput")
    wacc = nc.dram_tensor("wacc", [D, NWIN], f32, kind="ExternalOutput")
    eout = nc.dram_tensor("eout", [4, NCHUNK * G], bf16, kind="ExternalOutput")

    with TileContext(nc) as tc:
        with tc.tile_pool(name="const", bufs=1) as cpool, \
             tc.tile_pool(name="xtp", bufs=6) as xpool, \
             tc.tile_pool(name="thp", bufs=2) as thpool, \
             tc.tile_pool(name="junkp", bufs=2) as jpool, \
             tc.tile_pool(name="accp", bufs=1) as apool, \
             tc.tile_pool(name="psb", bufs=1, space="PSUM") as psb:

            w1sb = cpool.tile([D, A], bf16, tag="w1")
            w2rsb = cpool.tile([A, 128], bf16, tag="w2r")
            onesb = cpool.tile([128, 128], bf16, tag="ones1")
            b1sb = cpool.tile([A, 1], f32, tag="b1")
            b2sb = cpool.tile([128, 1], f32, tag="b2")

            def load_consts():
                nc.sync.dma_start(out=w1sb[:], in_=w1[:])
                nc.sync.dma_start(out=b1sb[:], in_=b1[:])
                nc.sync.dma_start(out=w2rsb[:], in_=w2r[:])
                nc.sync.dma_start(out=b2sb[:], in_=b2[:])
                nc.sync.dma_start(out=onesb[:], in_=ones1[:])

            waccsb = apool.tile([D, NWIN], f32, tag="wacc")
            nc.vector.memset(waccsb[:], 0.0)
            # Persistent store for all chunks' grouped e tiles (bf16, 31 KB
            # per partition); exported once at the end.
            eall = apool.tile([128, NCHUNK * G], bf16, tag="eall")
            nc.vector.memset(eall[:, 0:MM_N].bitcast(f32), 1.0)

            # One PSUM tensor spanning all 8 banks.  Layout per chunk:
            #   hregs = [0:1024], [1024:2048]   banks 0-1 / 2-3 (pre-tanh h,
            #           two half-chunk buffers so PE overlaps tanh)
            #   sgreg = [CHUNK:CHUNK+G]  bank 4, overwritten by ebc g0 after
            #           exp has consumed it (true-dep aligned)
            #   ebc   = [CHUNK:2*CHUNK]  banks 4-7 (broadcast e, fp32)
            pbig = psb.tile([128, 2 * CHUNK], f32, tag="pbig")
            HALF = CHUNK // 2
            hregs = [pbig[:, 0:HALF], pbig[:, HALF:CHUNK]]
            sgreg = pbig[:, CHUNK:CHUNK + G]
            ebc = pbig[:, CHUNK:2 * CHUNK]

            # Warm the PE's HAM clock gate during the initial DMA wait:
            # ~5 us of junk matmuls (inputs: the memset slice of eall; output:
            # the sgreg scratch, later cleared by score(0)'s start=True).
            for _ in range(8):
                nc.tensor.matmul(out=sgreg[:],
                                 lhsT=eall[:, 0:128],
                                 rhs=eall[:, 0:MM_N],
                                 start=True, stop=True)

            xtiles, ths = {}, {}

            def load_chunk(c, split=False):
                xtile = xpool.tile([D, CHUNK], bf16, tag="x")
                base = c * CHUNK
                if split:
                    # First chunk: sync/HWDGE quarter-loads issued before
                    # the const DMAs — skips the ~6us Q7 IRAM load the first
                    # SWDGE op pays, so the first h-matmul unblocks early.
                    for q in range(4):
                        nc.sync.dma_start(
                            out=xtile[:, q * MM_N:(q + 1) * MM_N],
                            in_=xt[:, base + q * MM_N:base + (q + 1) * MM_N])
                    load_consts()
                else:
                    nc.gpsimd.dma_start(out=xtile[:],
                                        in_=xt[:, base:base + CHUNK])
                xtiles[c] = xtile

            def stage_scores(p):
                # 4 col-tiled score matmuls: group g writes scores for rows
                # 512g..512g+511 onto partitions 32g..32g+31 (bank 4).
                th_p = ths[p]
                for g in range(4):
                    nc.tensor.matmul(
                        out=sgreg[32 * g:32 * (g + 1), :],
                        lhsT=w2rsb[:, 32 * g:32 * (g + 1)],
                        rhs=th_p[:, G * g:G * (g + 1)],
                        start=True, stop=True,
                        tile_position=(0, 32 * g))
                eg = eall[:, p * G:(p + 1) * G]
                nc.scalar.activation(out=eg, in_=sgreg,
                                     func=mybir.ActivationFunctionType.Exp,
                                     bias=b2sb[:, 0:1])

            def stage_h(c):
                # h matmuls + tanh in half-chunks so the PE can start the
                # next half/chunk while ACT runs tanh on the previous one.
                xtile = xtiles[c]
                th = thpool.tile([A, CHUNK], bf16, tag="th")
                ths[c] = th
                for hh in range(2):
                    off = hh * HALF
                    for i in range(HALF // MM_N):
                        nc.tensor.matmul(
                            out=hregs[hh][:, i * MM_N:(i + 1) * MM_N],
                            lhsT=w1sb[:],
                            rhs=xtile[:, off + i * MM_N:off + (i + 1) * MM_N],
                            start=True, stop=True)
                    nc.scalar.activation(
                        out=th[:, off:off + HALF],
                        in_=hregs[hh],
                        func=mybir.ActivationFunctionType.Tanh,
                        bias=b1sb[:, 0:1])

            def stage_tail(p):
                # Broadcast e via K=1 row-tiled ones-matmuls, then the
                # windowed weighted reductions.
                xtile_p = xtiles.pop(p)
                ths.pop(p)
                eg = eall[:, p * G:(p + 1) * G]
                for g in range(4):
                    nc.tensor.matmul(
                        out=ebc[:, G * g:G * (g + 1)],
                        lhsT=onesb[32 * g:32 * g + 1, :],
                        rhs=eg[32 * g:32 * g + 1, :],
                        start=True, stop=True,
                        tile_position=(32 * g, 0))
                for w in range(CHUNK // WIN):
                    gw = p * (CHUNK // WIN) + w
                    junk = jpool.tile([D, 1], f32, tag="junk")
                    nc.vector.affine_mul_reduce(
                        out=junk[:].to_broadcast([D, WIN]),
                        accum_out=waccsb[:, gw:gw + 1],
                        in0=xtile_p[:, w * WIN:(w + 1) * WIN],
                        in1=ebc[:, w * WIN:(w + 1) * WIN],
                        scale=1.0,
                        bias=0.0)

            load_chunk(0, split=True)
            for cc in (1, 2):
                if cc < NCHUNK:
                    load_chunk(cc)
            # Period structure: [ebc(c-1) + windows(c-1)] | [h(c) + tanh(c)]
            # | [score(c) + exp(c)].  The e-broadcast matmuls run first each
            # period (their inputs finished last period), the windows drain
            # on DVE mid-period, and score/exp land at the end — so no
            # cross-engine cycle spans more than one period.
            stage_h(0)
            stage_scores(0)
            for c in range(1, NCHUNK + 1):
                if c + 2 < NCHUNK:
                    load_chunk(c + 2)
                stage_tail(c - 1)
                if c < NCHUNK:
                    stage_h(c)
                    stage_scores(c)

            for g in range(4):
                nc.sync.dma_start(out=eout[g:g + 1, :],
                                  in_=eall[32 * g:32 * g + 1, :])
            nc.sync.dma_start(out=wacc[:], in_=waccsb[:])

    nc.compile()
    return nc


def _to_bf16(a):
    import ml_dtypes
    return np.asarray(a).astype(ml_dtypes.bfloat16)


def _run_device(xt_shards, W1, W2, b1, b2, trace=False):
    from concourse.bass_utils import run_bass_kernel_spmd
    import ml_dtypes

    if "prog" not in _prog_cache:
        _prog_cache["prog"] = _build_program()
    nc = _prog_cache["prog"]

    w1_in = np.ascontiguousarray(_to_bf16(W1))
    w2r_in = np.ascontiguousarray(
        _to_bf16(np.tile(np.asarray(W2, dtype=np.float32).reshape(A, 1),
                         (1, 128))))
    ones_in = np.ones((128, 128), dtype=ml_dtypes.bfloat16)
    b1_in = np.ascontiguousarray(np.asarray(b1, np.float32).reshape(A, 1))
    b2_in = np.full((128, 1), np.float32(b2), dtype=np.float32)

    in_maps = [{"xt": xt_shards[i], "w1": w1_in, "w2r": w2r_in,
                "ones1": ones_in, "b1": b1_in, "b2": b2_in}
               for i in range(NCORES)]
    res = run_bass_kernel_spmd(nc, in_maps, core_ids=list(range(NCORES)),
                               trace=trace)
    return res


def kernel(x, batch_index, W1, b1, W2, b2, _want_results=False, _trace=False):
    import ml_dtypes

    x = np.asarray(x, dtype=np.float32)
    bi64 = np.asarray(batch_index).astype(np.int64)
    b2v = float(np.asarray(b2, dtype=np.float32).reshape(-1)[0])

    assert x.shape == (N, D)

    # Host pre-transpose + bf16: xT [D, N] bf16, then per-core padded shards.
    xb = x.astype(ml_dtypes.bfloat16)
    xtb = xb.T
    xt_shards = []
    for i in range(NCORES):
        sh = np.zeros((D, RPAD), dtype=ml_dtypes.bfloat16)
        sh[:, :RPC] = xtb[:, i * RPC:(i + 1) * RPC]
        xt_shards.append(np.ascontiguousarray(sh))

    res = _run_device(xt_shards, W1, W2, b1, b2v, trace=_trace)

    # Gather device outputs.
    e = np.empty(N, dtype=np.float32)
    waccs = []
    for i in range(NCORES):
        eo = res.results[i]["eout"].astype(np.float32)
        # eout[g, p*G + m] holds e for row p*CHUNK + 512g + m: regroup to
        # chunk-major order.
        eo = eo.reshape(4, NCHUNK, G).transpose(1, 0, 2).reshape(-1)
        e[i * RPC:(i + 1) * RPC] = eo[:RPC]
        waccs.append(res.results[i]["wacc"])

    # Denominators: segment sums of e (same bf16 values the device used).
    denom = np.bincount(bi64, weights=e.astype(np.float64), minlength=B)

    # Numerators: pure windows from device sums; boundary windows recomputed.
    num = np.zeros((B, D), dtype=np.float64)
    for i in range(NCORES):
        wacc_i = waccs[i]
        base = i * RPC
        for w in range(NWIN):
            glo = base + w * WIN
            if glo >= base + RPC:
                break
            ghi = min(glo + WIN, base + RPC)
            b_first = bi64[glo]
            b_last = bi64[ghi - 1]
            if b_first == b_last:
                num[b_first] += wacc_i[:, w]
            else:
                sub = bi64[glo:ghi]
                cuts = np.flatnonzero(np.diff(sub)) + 1
                bounds = np.concatenate(([0], cuts, [ghi - glo]))
                for k in range(len(bounds) - 1):
                    lo, hi = glo + bounds[k], glo + bounds[k + 1]
                    num[sub[bounds[k]]] += \
                        e[lo:hi].astype(np.float64) @ x[lo:hi].astype(np.float64)

    dn = denom[:, None]
    out = np.divide(num, dn, out=np.zeros_like(num), where=dn > 0)
    out = out.astype(np.float32)
    if _want_results:
        return out, res
    return out


# revision 25
# speedup vs baseline: 1.0681x; 1.0681x over previous
"""Trainium2 Bass kernel for nn_AttentionLayer (segment softmax attention pooling).

Computation (reference):
    h = tanh(x @ W1 + b1)            # [N, A]
    s = h @ W2 + b2                  # [N, 1]
    per-segment softmax over s, out[b] = sum_i softmax_w_i * x_i   # [B, D]

Strategy (v2, bf16 data path + grouped exp):
  - Shard the N=500k instances across 8 NeuronCores (data parallel), weights
    replicated. Host pre-converts x to bf16 and pre-transposes so each core
    streams xT [D=128, rows] tiles (bf16 halves HBM traffic and doubles PE
    throughput vs the f32r baseline).
  - Per core, one pass over x, 2048-row chunks, software-pipelined:
      PE : hT = W1^T @ xT                     (4x N=512 matmuls, bf16)
      ACT: th = tanh(hT + b1) -> bf16
      PE : 4x col-tiled score matmuls -> sg [128, 512] grouped scores
           (col group g holds scores for rows 512g..512g+511 on partitions
           32g..32g+31; ACT cost is free-dim-driven, so exp on the grouped
           layout is 4x cheaper than on the broadcast layout)
      ACT: eg = exp(sg + b2) -> bf16 [128, 512]
      PE : 4x K=1 ones-matmuls broadcast eg group rows -> ebc [128, 2048]
           (PSUM) = e_i on every partition
      DVE: affine_mul_reduce(xT * ebc) summed per 1024-row window -> wacc
  - Device outputs: per-window weighted sums wacc [D, NWIN] and the e row
    (bf16, via the grouped eg tile).  Segment logic on the host: pure windows
    used directly; windows containing a segment boundary recomputed from x
    and the exported e; denominators via bincount over e.  exp without max
    subtraction is safe (scores O(+-5)); numerator/denominator share e.
"""

import numpy as np

# Problem constants (hardcoded per contract; kernel.py must be self-contained).
N = 500_000
D = 128
A = 128
B = 256
NCORES = 8
RPC = N // NCORES            # rows per core = 62500
CHUNK = 2048                 # rows per streamed tile
WIN = 1024                   # rows per reduction window
G = CHUNK // 4               # grouped score tile free dim = 512
NCHUNK = -(-RPC // CHUNK)    # 31
RPAD = NCHUNK * CHUNK        # 63488
NWIN = RPAD // WIN           # 62
MM_N = 512                   # PE moving-operand max free dim

_prog_cache = {}


def _build_program():
    import concourse.bacc as bacc
    from concourse import mybir
    from concourse.tile import TileContext

    f32 = mybir.dt.float32
    bf16 = mybir.dt.bfloat16
    nc = bacc.Bacc("TRN2", target_bir_lowering=False, debug=False,
                   num_devices=NCORES)

    xt = nc.dram_tensor("xt", [D, RPAD], bf16, kind="ExternalInput")
    w1 = nc.dram_tensor("w1", [D, A], bf16, kind="ExternalInput")
    w2r = nc.dram_tensor("w2r", [A, 128], bf16, kind="ExternalInput")
    ones1 = nc.dram_tensor("ones1", [128, 128], bf16, kind="ExternalInput")
    b1 = nc.dram_tensor("b1", [A, 1], f32, kind="ExternalInput")
    b2 = nc.dram_tensor("b2", [128, 1], f32, kind="ExternalInput")
    wacc = nc.dram_tensor("wacc", [D, NWIN], f32, kind="ExternalOutput")
    eout = nc.dram_tensor("eout", [4, NCHUNK * G], bf16, kind="ExternalOutput")

    with TileContext(nc) as tc:
        with tc.tile_pool(name="const", bufs=1) as cpool, \
             tc.tile_pool(name="xtp", bufs=6) as xpool, \
             tc.tile_pool(name="thp", bufs=2) as thpool, \
             tc.tile_pool(name="junkp", bufs=2) as jpool, \
             tc.tile_pool(name="accp", bufs=1) as apool, \
             tc.tile_pool(name="psb", bufs=1, space="PSUM") as psb:

            w1sb = cpool.tile([D, A], bf16, tag="w1")
            w2rsb = cpool.tile([A, 128], bf16, tag="w2r")
            onesb = cpool.tile([128, 128], bf16, tag="ones1")
            b1sb = cpool.tile([A, 1], f32, tag="b1")
            b2sb = cpool.tile([128, 1], f32, tag="b2")

            nc.sync.dma_start(out=w1sb[:], in_=w1[:])
            nc.sync.dma_start(out=b1sb[:], in_=b1[:])
            nc.sync.dma_start(out=w2rsb[:], in_=w2r[:])
            nc.sync.dma_start(out=b2sb[:], in_=b2[:])
            nc.sync.dma_start(out=onesb[:], in_=ones1[:])

            waccsb = apool.tile([D, NWIN], f32, tag="wacc")
            nc.vector.memset(waccsb[:], 0.0)
            # Persistent store for all chunks' grouped e tiles (bf16, 31 KB
            # per partition); exported once at the end.
            eall = apool.tile([128, NCHUNK * G], bf16, tag="eall")
            nc.vector.memset(eall[:, 0:MM_N].bitcast(f32), 1.0)

            # One PSUM tensor spanning all 8 banks.  Layout per chunk:
            #   hregs = [0:1024], [1024:2048]   banks 0-1 / 2-3 (pre-tanh h,
            #           two half-chunk buffers so PE overlaps tanh)
            #   sgreg = [CHUNK:CHUNK+G]  bank 4, overwritten by ebc g0 after
            #           exp has consumed it (true-dep aligned)
            #   ebc   = [CHUNK:2*CHUNK]  banks 4-7 (broadcast e, fp32)
            pbig = psb.tile([128, 2 * CHUNK], f32, tag="pbig")
            HALF = CHUNK // 2
            hregs = [pbig[:, 0:HALF], pbig[:, HALF:CHUNK]]
            sgreg = pbig[:, CHUNK:CHUNK + G]
            ebc = pbig[:, CHUNK:2 * CHUNK]

            # Warm the PE's HAM clock gate during the initial DMA wait:
            # ~5 us of junk matmuls (inputs: the memset slice of eall; output:
            # the sgreg scratch, later cleared by score(0)'s start=True).
            for _ in range(8):
                nc.tensor.matmul(out=sgreg[:],
                                 lhsT=eall[:, 0:128],
                                 rhs=eall[:, 0:MM_N],
                                 start=True, stop=True)

            xtiles, ths = {}, {}

            def load_chunk(c, split=False):
                xtile = xpool.tile([D, CHUNK], bf16, tag="x")
                base = c * CHUNK
                if split:
                    # First chunk: quarter-loads so the first h-matmul
                    # unblocks after 128 KB instead of 512 KB.
                    for q in range(4):
                        nc.gpsimd.dma_start(
                            out=xtile[:, q * MM_N:(q + 1) * MM_N],
                            in_=xt[:, base + q * MM_N:base + (q + 1) * MM_N])
                else:
                    nc.gpsimd.dma_start(out=xtile[:],
                                        in_=xt[:, base:base + CHUNK])
                xtiles[c] = xtile

            def stage_scores(p):
                # 4 col-tiled score matmuls: group g writes scores for rows
                # 512g..512g+511 onto partitions 32g..32g+31 (bank 4).
                th_p = ths[p]
                for g in range(4):
                    nc.tensor.matmul(
                        out=sgreg[32 * g:32 * (g + 1), :],
                        lhsT=w2rsb[:, 32 * g:32 * (g + 1)],
                        rhs=th_p[:, G * g:G * (g + 1)],
                        start=True, stop=True,
                        tile_position=(0, 32 * g))
                eg = eall[:, p * G:(p + 1) * G]
                nc.scalar.activation(out=eg, in_=sgreg,
                                     func=mybir.ActivationFunctionType.Exp,
                                     bias=b2sb[:, 0:1])

            def stage_h(c):
                # h matmuls + tanh in half-chunks so the PE can start the
                # next half/chunk while ACT runs tanh on the previous one.
                xtile = xtiles[c]
                th = thpool.tile([A, CHUNK], bf16, tag="th")
                ths[c] = th
                for hh in range(2):
                    off = hh * HALF
                    for i in range(HALF // MM_N):
                        nc.tensor.matmul(
                            out=hregs[hh][:, i * MM_N:(i + 1) * MM_N],
                            lhsT=w1sb[:],
                            rhs=xtile[:, off + i * MM_N:off + (i + 1) * MM_N],
                            start=True, stop=True)
                    nc.scalar.activation(
                        out=th[:, off:off + HALF],
                        in_=hregs[hh],
                        func=mybir.ActivationFunctionType.Tanh,
                        bias=b1sb[:, 0:1])

            def stage_tail(p):
                # Broadcast e via K=1 row-tiled ones-matmuls, then the
                # windowed weighted reductions.
                xtile_p = xtiles.pop(p)
                ths.pop(p)
                eg = eall[:, p * G:(p + 1) * G]
                for g in range(4):
                    nc.tensor.matmul(
                        out=ebc[:, G * g:G * (g + 1)],
                        lhsT=onesb[32 * g:32 * g + 1, :],
                        rhs=eg[32 * g:32 * g + 1, :],
                        start=True, stop=True,
                        tile_position=(32 * g, 0))
                for w in range(CHUNK // WIN):
                    gw = p * (CHUNK // WIN) + w
                    junk = jpool.tile([D, 1], f32, tag="junk")
                    nc.vector.affine_mul_reduce(
                        out=junk[:].to_broadcast([D, WIN]),
                        accum_out=waccsb[:, gw:gw + 1],
                        in0=xtile_p[:, w * WIN:(w + 1) * WIN],
                        in1=ebc[:, w * WIN:(w + 1) * WIN],
                        scale=1.0,
                        bias=0.0)

            load_chunk(0, split=True)
            for cc in (1, 2):
                if cc < NCHUNK:
                    load_chunk(cc)
            # Period structure: [ebc(c-1) + windows(c-1)] | [h(c) + tanh(c)]
            # | [score(c) + exp(c)].  The e-broadcast matmuls run first each
            # period (their inputs finished last period), the windows drain
            # on DVE mid-period, and score/exp land at the end — so no
            # cross-engine cycle spans more than one period.
            stage_h(0)
            stage_scores(0)
            for c in range(1, NCHUNK + 1):
                if c + 2 < NCHUNK:
                    load_chunk(c + 2)
                stage_tail(c - 1)
                if c < NCHUNK:
                    stage_h(c)
                    stage_scores(c)

            for g in range(4):
                nc.sync.dma_start(out=eout[g:g + 1, :],
                                  in_=eall[32 * g:32 * g + 1, :])
            nc.sync.dma_start(out=wacc[:], in_=waccsb[:])

    nc.compile()
    return nc


def _to_bf16(a):
    import ml_dtypes
    return np.asarray(a).astype(ml_dtypes.bfloat16)


def _run_device(xt_shards, W1, W2, b1, b2, trace=False):
    from concourse.bass_utils import run_bass_kernel_spmd
    import ml_dtypes

    if "prog" not in _prog_cache:
        _prog_cache["prog"] = _build_program()
    nc = _prog_cache["prog"]

    w1_in = np.ascontiguousarray(_to_bf16(W1))
    w2r_in = np.ascontiguousarray(
        _to_bf16(np.tile(np.asarray(W2, dtype=np.float32).reshape(A, 1),
                         (1, 128))))
    ones_in = np.ones((128, 128), dtype=ml_dtypes.bfloat16)
    b1_in = np.ascontiguousarray(np.asarray(b1, np.float32).reshape(A, 1))
    b2_in = np.full((128, 1), np.float32(b2), dtype=np.float32)

    in_maps = [{"xt": xt_shards[i], "w1": w1_in, "w2r": w2r_in,
                "ones1": ones_in, "b1": b1_in, "b2": b2_in}
               for i in range(NCORES)]
    res = run_bass_kernel_spmd(nc, in_maps, core_ids=list(range(NCORES)),
                               trace=trace)
    return res


def kernel(x, batch_index, W1, b1, W2, b2, _want_results=False, _trace=False):
    import ml_dtypes

    x = np.asarray(x, dtype=np.float32)
    bi64 = np.asarray(batch_index).astype(np.int64)
    b2v = float(np.asarray(b2, dtype=np.float32).reshape(-1)[0])

    assert x.shape == (N, D)

    # Host pre-transpose + bf16: xT [D, N] bf16, then per-core padded shards.
    xb = x.astype(ml_dtypes.bfloat16)
    xtb = xb.T
    xt_shards = []
    for i in range(NCORES):
        sh = np.zeros((D, RPAD), dtype=ml_dtypes.bfloat16)
        sh[:, :RPC] = xtb[:, i * RPC:(i + 1) * RPC]
        xt_shards.append(np.ascontiguousarray(sh))

    res = _run_device(xt_shards, W1, W2, b1, b2v, trace=_trace)

    # Gather device outputs.
    e = np.empty(N, dtype=np.float32)
    waccs = []
    for i in range(NCORES):
        eo = res.results[i]["eout"].astype(np.float32)
        # eout[g, p*G + m] holds e for row p*CHUNK + 512g + m: regroup to
        # chunk-major order.
        eo = eo.reshape(4, NCHUNK, G).transpose(1, 0, 2).reshape(-1)
        e[i * RPC:(i + 1) * RPC] = eo[:RPC]
        waccs.append(res.results[i]["wacc"])

    # Denominators: segment sums of e (same bf16 values the device used).
    denom = np.bincount(bi64, weights=e.astype(np.float64), minlength=B)

    # Numerators: pure windows from device sums; boundary windows recomputed.
    num = np.zeros((B, D), dtype=np.float64)
    for i in range(NCORES):
        wacc_i = waccs[i]
        base = i * RPC
        for w in range(NWIN):
            glo = base + w * WIN
            if glo >= base + RPC:
                break
            ghi = min(glo + WIN, base + RPC)
            b_first = bi64[glo]
            b_last = bi64[ghi - 1]
            if b_first == b_last:
                num[b_first] += wacc_i[:, w]
            else:
                sub = bi64[glo:ghi]
                cuts = np.flatnonzero(np.diff(sub)) + 1
                bounds = np.concatenate(([0], cuts, [ghi - glo]))
                for k in range(len(bounds) - 1):
                    lo, hi = glo + bounds[k], glo + bounds[k + 1]
                    num[sub[bounds[k]]] += \
                        e[lo:hi].astype(np.float64) @ x[lo:hi].astype(np.float64)

    dn = denom[:, None]
    out = np.divide(num, dn, out=np.zeros_like(num), where=dn > 0)
    out = out.astype(np.float32)
    if _want_results:
        return out, res
    return out
